# revision 2
# baseline (speedup 1.0000x reference)
"""
Trainium2 Bass kernel for nn_LinearLUT (residual-binarized LUT linear layer).

Math restructure
----------------
reference(x) computes, per sample b and per table t (t = o*128 + j, one table
per (out_feature o, in_feature j)):

  table_out[b,t] = sum_l f_t(m_l * s_l[b, idx_1(t)], ..., m_l * s_l[b, idx_4(t)])

where f_t is the multilinear (Lagrange) interpolation of the 16-entry LUT
weight[t, :] on {-1,+1}^4, s_l are the level-l sign bits of x, and
idx_i(t) = input_mask[t*4+i].  Since every argument is +-m_l, f_t only
depends on the 4 sign bits => precompute (host, weight-static):

  Q_l[t, v] = sum_c weight[t,c] * prod_i (1 + m_l*sig(v,i)*tt(c,i))/2

a 16-entry lookup per (t, level), indexed by the 4-bit sign code
  code_l[b,t] = sum_i 2^i * bit_l[b, idx_i(t)]  =  (bit_l @ G)[b,t]
with G[j,t] = sum_i 2^i [idx_i(t)==j]  -- ONE matmul per level.

On device (per core; tables sharded 8 ways, T_c=2048 tables = 16 out
features per core):
  1. sign bits from xT (DVE, 3 small ops)
  2. codeT[t_p,(tile,b)] = G_tile^T @ bitT   (PE, 32 matmuls)
  3. PSUM->SBUF fp16 copies (ACT)
  4. one-hot planes eq_v = (codeT == v)      (DVE tensor_scalar, 32 ops)
  5. LUT-select + per-out-feature segment-sum fused into PE:
       y[b, o] += eq_{l,v}[:, tile-slice]^T @ Qcol_{l,tile,v}   (N=1 matmuls,
     PSUM-accumulated over l,v; j-contraction does the segment sum)
  6. copy y PSUM->SBUF, DMA out [128, 16] f32; host concatenates cores.
"""

import numpy as np

import concourse.bass as bass
import concourse.bacc as bacc
import concourse.mybir as mybir
import concourse.tile as tile
from concourse.bass_utils import run_bass_kernel_spmd

# Problem dims (hardcoded per contract)
LEVELS = 2
K = 4
KK = 16
IN = 128
OUT = 128
B = 128
T = IN * OUT  # 16384
NCORES = 8
T_C = T // NCORES     # 2048 tables per core
OL = OUT // NCORES    # 16 out features per core
NTILE = T_C // 128    # 16 t-tiles per core

F16 = mybir.dt.float16
F32 = mybir.dt.float32

_CACHED_NC = None


def _build_nc():
    """Build the per-core Bass program (identical on all 8 cores)."""
    nc = bacc.Bacc("TRN2", target_bir_lowering=False, debug=False,
                   num_devices=NCORES)

    NV = KK - 1  # v=0 dropped; handled via const rank-1 matmul
    xt = nc.dram_tensor("xt", [IN, B], F32, kind="ExternalInput")
    consts = nc.dram_tensor("consts", [128, 2], F32, kind="ExternalInput")
    g = nc.dram_tensor("g", [IN, T_C], F16, kind="ExternalInput")
    qcols = nc.dram_tensor("qcols", [128, LEVELS * NTILE * NV], F16,
                           kind="ExternalInput")
    cvec = nc.dram_tensor("cvec", [1, OL], F32, kind="ExternalInput")
    y = nc.dram_tensor("y", [B, OL], F32, kind="ExternalOutput")

    NQ = NTILE // 4  # psum chunks per level (4 t-tiles each)

    with tile.TileContext(nc) as tc:
        with (
            tc.tile_pool(name="const", bufs=1) as cpool,
            tc.tile_pool(name="bits", bufs=1) as bpool,
            tc.tile_pool(name="codesb", bufs=1) as csbpool,
            tc.tile_pool(name="eq", bufs=6) as eqpool,
            tc.tile_pool(name="out", bufs=1) as opool,
            tc.tile_pool(name="psum_code", bufs=7,
                         space=bass.MemorySpace.PSUM) as pc,
            tc.tile_pool(name="psum_y", bufs=1,
                         space=bass.MemorySpace.PSUM) as py,
        ):
            xt_sb = cpool.tile([IN, B], F32, tag="xt")
            c_sb = cpool.tile([128, 2], F32, tag="consts")
            cv_sb = cpool.tile([1, OL], F32, tag="cvec")
            ones_sb = cpool.tile([1, B], F32, tag="ones")
            nc.sync.dma_start(xt_sb[:], xt[:])
            nc.sync.dma_start(c_sb[:], consts[:])
            nc.sync.dma_start(cv_sb[:], cvec[:])
            nc.gpsimd.memset(ones_sb[:], 1.0)
            # g in 4 chunks so code matmuls start before the full DMA lands
            g_sb = []
            for q in range(NQ):
                gq = cpool.tile([IN, 4 * 128], F16, tag=f"g{q}",
                                name=f"g_sb{q}")
                nc.sync.dma_start(gq[:], g[:, q * 512:(q + 1) * 512])
                g_sb.append(gq)
            q_sb = cpool.tile([128, LEVELS * NTILE * NV], F16, tag="qcols")
            nc.sync.dma_start(q_sb[:], qcols[:])

            # ---- sign bits (as fp16 0/1, j on partitions) ----
            bit1 = bpool.tile([IN, B], F16, tag="bit1")
            nc.vector.tensor_scalar(bit1[:], xt_sb[:], 0.0, None,
                                    mybir.AluOpType.is_ge)
            # rc = x - 2*m0*bit1   (== resid - m0)
            rc = bpool.tile([IN, B], F32, tag="rc")
            nc.vector.scalar_tensor_tensor(rc[:], bit1[:], c_sb[:, 0:1],
                                           xt_sb[:], mybir.AluOpType.mult,
                                           mybir.AluOpType.add)
            # bit2 = (rc >= -m0)
            bit2 = bpool.tile([IN, B], F16, tag="bit2")
            nc.vector.tensor_scalar(bit2[:], rc[:], c_sb[:, 1:2], None,
                                    mybir.AluOpType.is_ge)
            bits = [bit1, bit2]

            # ---- code matmuls + PSUM->SBUF fp16 copies ----
            # codesb_l[t_p, (tile, b)], one [128, 2048] fp16 tensor per level
            codesb = [csbpool.tile([128, NTILE * B], F16, tag=f"code{l}",
                                   name=f"codesb{l}")
                      for l in range(LEVELS)]
            cps_tiles = {}
            for l in range(LEVELS):
                for q in range(NQ):
                    cps = pc.tile([128, 4 * B], F32, tag="codepsum",
                                  name=f"cps{l}_{q}")
                    cps_tiles[(l, q)] = cps
                    for k in range(4):
                        t_i = q * 4 + k
                        nc.tensor.matmul(
                            cps[:, k * B:(k + 1) * B],
                            g_sb[q][:, k * 128:(k + 1) * 128],
                            bits[l][:],
                            start=True, stop=True,
                        )
                    # level-0 copies split ACT/DVE to cut eq start latency;
                    # level-1 copies all on ACT (DVE busy with eq by then)
                    dst = codesb[l][:, q * 4 * B:(q + 1) * 4 * B]
                    if l == 0 and q >= 2:
                        nc.vector.tensor_copy(dst, cps[:])
                    else:
                        nc.scalar.copy(dst, cps[:])

            # ---- one-hot planes + fused LUT/segment-sum matmuls ----
            y_ps = py.tile([B, OL], F32, tag="ypsum")
            # rank-1 seed: y[b, o] = cvec[o]  (bias + sum_j Q_l[t,0] terms)
            nc.tensor.matmul(y_ps[:], ones_sb[:], cv_sb[:],
                             start=True, stop=False)
            n_acc = LEVELS * NV  # accumulation steps per column after seed
            for l in range(LEVELS):
                for v in range(1, KK):
                    eq = eqpool.tile([128, NTILE * B], F16, tag="eq")
                    nc.vector.tensor_scalar(eq[:], codesb[l][:], float(v),
                                            None, mybir.AluOpType.is_equal)
                    step = l * NV + (v - 1)
                    for t_i in range(NTILE):
                        col = (l * NTILE + t_i) * NV + (v - 1)
                        nc.tensor.matmul(
                            y_ps[:, t_i:t_i + 1],
                            eq[:, t_i * B:(t_i + 1) * B],
                            q_sb[:, col:col + 1],
                            start=False,
                            stop=(step == n_acc - 1 and t_i == NTILE - 1),
                        )

            y_sb = opool.tile([B, OL], F32, tag="ysb")
            nc.scalar.copy(y_sb[:], y_ps[:])
            nc.sync.dma_start(y[:], y_sb[:])

    nc.compile()
    return nc


def _host_prep(x, weight, bias, means):
    """Weight-static preprocessing: Q LUTs, G matrix, consts, xT."""
    w = weight.astype(np.float64)
    m = np.abs(means.astype(np.float64))
    cc = np.arange(KK)
    tt = (2 * ((cc[:, None] >> np.arange(K)[None, :]) & 1) - 1).astype(
        np.float64)          # [c, i]
    sig = tt                  # same construction for sign patterns [v, i]

    qs = []
    for l in range(LEVELS):
        # M[v, c] = prod_i (1 + m_l * sig[v,i] * tt[c,i]) / 2
        M = np.prod((1.0 + m[l] * sig[:, None, :] * tt[None, :, :]) * 0.5,
                    axis=-1)  # [v, c]
        q = w @ M.T           # [T, KK]
        qs.append(q)
    return qs


def _build_g(input_mask):
    G = np.zeros((IN, T), np.float64)
    cols = np.repeat(np.arange(T), K)
    vals = np.tile(2.0 ** np.arange(K), T)
    np.add.at(G, (input_mask.astype(np.int64), cols), vals)
    return G


def _make_in_maps(x, weight, bias, means, input_mask):
    qs = _host_prep(x, weight, bias, means)
    G = _build_g(input_mask)

    m0 = float(np.abs(means.astype(np.float64))[0])
    consts = np.zeros((128, 2), np.float32)
    consts[:, 0] = -2.0 * m0
    consts[:, 1] = -m0
    xt = np.ascontiguousarray(x.astype(np.float32).T)

    # const[o] = bias[o] + sum_l sum_j Q_l[o*IN+j, 0]
    cvec_full = bias.astype(np.float64).copy()
    for l in range(LEVELS):
        cvec_full += qs[l][:, 0].reshape(OUT, IN).sum(-1)

    in_maps = []
    NV = KK - 1
    for c in range(NCORES):
        t0 = c * T_C
        gc = G[:, t0:t0 + T_C].astype(np.float16)
        # qcols[j, (l, tile, v-1)] = Q_l[t0 + tile*128 + j, v] - Q_l[., 0]
        qc = np.empty((128, LEVELS, NTILE, NV), np.float16)
        for l in range(LEVELS):
            dq = qs[l][t0:t0 + T_C, 1:] - qs[l][t0:t0 + T_C, 0:1]
            qc[:, l] = dq.reshape(NTILE, 128, NV).transpose(1, 0, 2)
        in_maps.append({
            "xt": xt,
            "consts": consts,
            "g": np.ascontiguousarray(gc),
            "qcols": np.ascontiguousarray(qc.reshape(128, -1)),
            "cvec": np.ascontiguousarray(
                cvec_full[c * OL:(c + 1) * OL].astype(np.float32)[None, :]),
        })
    return in_maps


def kernel(x, weight, bias, means, input_mask):
    global _CACHED_NC
    if _CACHED_NC is None:
        _CACHED_NC = _build_nc()
    nc = _CACHED_NC

    in_maps = _make_in_maps(x, weight, bias, means, input_mask)
    res = run_bass_kernel_spmd(nc, in_maps, list(range(NCORES)))
    globals()["_LAST_RESULTS"] = res
    out = np.concatenate([res.results[c]["y"] for c in range(NCORES)], axis=1)
    return out.astype(np.float32)

